# revision 32
# baseline (speedup 1.0000x reference)
"""Causal multi-head attention on 8 Trainium2 NeuronCores.

Sharding: core c -> batch (c // 4), head-group (c % 4) of 4 heads
(tensor-parallel over the 16 heads, data-parallel over batch=2).
Each core computes its 4 heads' contribution to the output projection;
the host sums the 4 per-head-group partials per batch (the "all-reduce")
and adds b_O.

Kernel layout notes (per core):
  - everything is computed in transposed [feature, seq] layout so the
    softmax reduction (over keys) lands on PSUM partitions and can be
    done with a ones-matmul on the PE.
  - QK^T uses float32r matmuls (full fp32 data, 1 cycle/row at N=512).
  - softmax weights (exp scores) are bf16; numerator (P@V) and
    denominator (ones-matmul) use the same bf16 values, so the
    normalization is self-consistent.
  - b_K/b_V/b_Q are folded in exactly via a K=1 "augmented feature"
    matmul that is only emitted when any bias is nonzero; b_O and the
    partial sum over head-groups happen on the host.
"""

import os
import sys

for _p in ("/opt/trn_rl_repo", "/root/.axon_site/_ro/trn_rl_repo"):
    if os.path.isdir(_p) and _p not in sys.path:
        sys.path.append(_p)

import ml_dtypes
import numpy as np

import concourse.bacc as bacc
import concourse.mybir as mybir
import concourse.tile as tile
from concourse.bass_utils import run_bass_kernel_spmd

F32 = mybir.dt.float32
F32R = mybir.dt.float32r
BF16 = mybir.dt.bfloat16

B = 2          # batch
S = 2048       # sequence length
DM = 1024      # d_model
DH = 64        # d_head
NHEAD = 16     # total heads
NH = 4         # heads per core
NPAIR = 2      # head pairs per core
DC = DM // 128   # d_model chunks of 128 -> 8
KC = S // 128    # key chunks of 128 -> 16
QT = S // 512    # query tiles of 512 -> 4
NEG = -1.0e30

# Set by test harness to capture HW profile; harmless defaults for grading.
TRACE = False
TRACE_DIR = None
LAST_EXEC_NS = None


def _build(with_bias: bool):
    nc = bacc.Bacc("TRN2", target_bir_lowering=False, debug=False)

    xT = nc.dram_tensor("xT", [DM, S], BF16, kind="ExternalInput").ap()
    wq = nc.dram_tensor("wq", [128, DC * NH * DH], BF16, kind="ExternalInput").ap()
    wk = nc.dram_tensor("wk", [128, DC * NH * DH], BF16, kind="ExternalInput").ap()
    wv = nc.dram_tensor("wv", [128, DC * NH * DH], BF16, kind="ExternalInput").ap()
    wo = nc.dram_tensor("wo", [128, NPAIR * DM], BF16, kind="ExternalInput").ap()
    mask = nc.dram_tensor("mask", [128, 128], BF16, kind="ExternalInput").ap()
    if with_bias:
        bq = nc.dram_tensor("bq", [1, NH * DH], BF16, kind="ExternalInput").ap()
        bk = nc.dram_tensor("bk", [1, NH * DH], BF16, kind="ExternalInput").ap()
        bv = nc.dram_tensor("bv", [1, NH * DH], BF16, kind="ExternalInput").ap()
    outT = nc.dram_tensor("outT", [DM, S], F32, kind="ExternalOutput").ap()

    with tile.TileContext(nc) as tc:
        with (
            tc.tile_pool(name="const", bufs=1) as cpool,
            tc.tile_pool(name="qk", bufs=1) as qkpool,
            tc.tile_pool(name="xt", bufs=8) as xtpool,
            tc.tile_pool(name="expS", bufs=2) as epool,
            tc.tile_pool(name="small", bufs=2) as spool,
            tc.tile_pool(name="zt", bufs=4) as ztpool,
            tc.tile_pool(name="out", bufs=3) as opool,
            tc.tile_pool(name="ps", bufs=1, space="PSUM") as psP,
        ):
            wo_sb = cpool.tile([128, NPAIR, DM], BF16, name="wo")
            mask_sb = cpool.tile([128, 128], BF16, name="mask")
            ones_bf = cpool.tile([128, DH], BF16, name="ones_bf")
            nc.vector.memset(ones_bf[:, :], 1.0)
            wq_sb = cpool.tile([128, DC, NH * DH], BF16, name="wq")
            wk_sb = cpool.tile([128, DC, NH * DH], BF16, name="wk")
            wv_sb = cpool.tile([128, DC, NH * DH], BF16, name="wv")
            if with_bias:
                ones32 = cpool.tile([128, 512], BF16, name="ones32")
                nc.vector.memset(ones32[:, :], 1.0)
                bq_sb = cpool.tile([128, NH * DH], BF16, name="bq")
                bk_sb = cpool.tile([128, NH * DH], BF16, name="bk")
                bv_sb = cpool.tile([128, NH * DH], BF16, name="bv")
                nc.sync.dma_start(bq_sb[0:1, :], bq[:, :])
                nc.sync.dma_start(bk_sb[0:1, :], bk[:, :])
                nc.sync.dma_start(bv_sb[0:1, :], bv[:, :])

            qt_sb = [qkpool.tile([128, S], BF16, name=f"qt{p}") for p in range(NPAIR)]
            kt_sb = [qkpool.tile([128, S], BF16, name=f"kt{p}") for p in range(NPAIR)]
            v_sb = qkpool.tile([128, KC, NH * DH], BF16, name="v")

            xt = []
            for c in range(DC):
                t = xtpool.tile([128, S], BF16, name="xt")
                eng = nc.sync if c % 2 == 0 else nc.scalar
                eng.dma_start(t[:, :], xT[c * 128:(c + 1) * 128, :])
                xt.append(t)
                if c == 0:
                    nc.scalar.dma_start(wq_sb[:, :, :], wq[:, :])
                    nc.scalar.dma_start(wk_sb[:, :, :], wk[:, :])
            nc.scalar.dma_start(wv_sb[:, :, :], wv[:, :])
            nc.sync.dma_start(wo_sb[:, :, :], wo[:, :])
            nc.sync.dma_start(mask_sb[:, :], mask[:, :])

            def qk_proj(p, qts):
                """Project Q and K for pair p over query tiles qts (c-outer)."""
                accs = {}
                for pj in range(2):
                    for q in qts:
                        accs[(pj, q)] = psP.tile([128, 512], F32, name="ps_acc", bufs=4)
                for c in range(DC):
                    for (pj, q), ps in accs.items():
                        w_sb = wq_sb if pj == 0 else wk_sb
                        nc.tensor.matmul(
                            ps[:, :],
                            lhsT=w_sb[:, c, p * 128:(p + 1) * 128],
                            rhs=xt[c][:, q * 512:(q + 1) * 512],
                            start=(c == 0),
                            stop=(c == DC - 1 and not with_bias),
                        )
                if with_bias:
                    for (pj, q), ps in accs.items():
                        bias_t = bq_sb if pj == 0 else bk_sb
                        nc.tensor.matmul(
                            ps[:, :],
                            lhsT=bias_t[0:1, p * 128:(p + 1) * 128],
                            rhs=ones32[0:1, :],
                            start=False,
                            stop=True,
                        )
                for (pj, q), ps in accs.items():
                    dst = qt_sb[p] if pj == 0 else kt_sb[p]
                    nc.vector.tensor_copy(dst[:, q * 512:(q + 1) * 512], ps[:, :])

            def v_proj(kts):
                """Project V (natural layout) for key chunks kts; 1 bank each."""
                accs = [psP.tile([128, 512], F32, name="ps_acc", bufs=4) for _ in kts]
                for c in range(DC):
                    for i, k in enumerate(kts):
                        nc.tensor.matmul(
                            accs[i][:, :NH * DH],
                            lhsT=xt[c][:, k * 128:(k + 1) * 128],
                            rhs=wv_sb[:, c, :],
                            start=(c == 0),
                            stop=(c == DC - 1 and not with_bias),
                            skip_group_check=True,
                        )
                if with_bias:
                    for i in range(len(kts)):
                        nc.tensor.matmul(
                            accs[i][:, :NH * DH],
                            lhsT=ones32[0:1, 0:128],
                            rhs=bv_sb[0:1, :],
                            start=False,
                            stop=True,
                            skip_group_check=True,
                        )
                for i, k in enumerate(kts):
                    nc.vector.tensor_copy(v_sb[:, k, :], accs[i][:, :NH * DH])

            zts = {}  # (p, j) -> zt tile

            def attn(p, j):
                nck = 4 * (j + 1)
                es = epool.tile([128, KC * 2 * 512], BF16, name="es")
                for c in range(nck):
                    tp = c - 4 * j
                    a = 128 * tp if tp >= 0 else 0
                    ps = psP.tile([128, 1024], F32, name="ps_sc", bufs=2)
                    for hi in range(2):
                        prow = slice(64 * hi, 64 * hi + 64)
                        nc.tensor.matmul(
                            ps[:, 512 * hi + a:512 * (hi + 1)],
                            lhsT=kt_sb[p][prow, c * 128:(c + 1) * 128],
                            rhs=qt_sb[p][prow, j * 512 + a:(j + 1) * 512],
                            start=True,
                            stop=True,
                        )
                    nc.scalar.activation(
                        es[:, c * 1024 + a:(c + 1) * 1024],
                        ps[:, a:],
                        mybir.ActivationFunctionType.Exp,
                    )
                    if tp >= 0:
                        for hi in range(2):
                            sl = slice((c * 2 + hi) * 512 + a,
                                       (c * 2 + hi) * 512 + a + 128)
                            nc.vector.tensor_mul(
                                out=es[:, sl], in0=es[:, sl], in1=mask_sb[:, :],
                            )
                # PV + column sums; h0/h1 col-packed, emitted adjacently so
                # they run concurrently in disjoint array column groups
                ps_z2 = psP.tile([128, 512], F32, name="ps_acc", bufs=4)
                ps_s2 = psP.tile([128, 512], F32, name="ps_acc", bufs=4)
                ps_z = [ps_z2, ps_z2]
                ps_s = [ps_s2, ps_s2]
                for c in range(nck):
                    tp = c - 4 * j
                    a = 128 * tp if tp >= 0 else 0
                    for hi in range(2):
                        col = 64 * hi
                        hcore = 2 * p + hi
                        nc.tensor.matmul(
                            ps_z[hi][col:col + 64, a:512],
                            lhsT=v_sb[:, c, hcore * DH:(hcore + 1) * DH],
                            rhs=es[:, (c * 2 + hi) * 512 + a:(c * 2 + hi + 1) * 512],
                            start=(c == 0),
                            stop=(c == nck - 1),
                            tile_position=(0, col),
                            skip_group_check=True,
                        )
                    for hi in range(2):
                        col = 64 * hi
                        nc.tensor.matmul(
                            ps_s[hi][col:col + 64, a:512],
                            lhsT=ones_bf[:, :],
                            rhs=es[:, (c * 2 + hi) * 512 + a:(c * 2 + hi + 1) * 512],
                            start=(c == 0),
                            stop=(c == nck - 1),
                            tile_position=(0, col),
                            skip_group_check=True,
                        )
                recip = spool.tile([128, 512], F32, name="recip")
                nc.vector.reciprocal_approx_fast(recip[:, :], ps_s2[:, :])
                zt = ztpool.tile([128, 512], BF16, name=f"zt{p}")
                nc.vector.tensor_mul(zt[:, :], ps_z2[:, :], recip[:, :])
                zts[(p, j)] = zt

            def emit_wo(j):
                for d in range(DC):
                    ps = psP.tile([128, 512], F32, name="ps_acc", bufs=4)
                    for p in range(NPAIR):
                        nc.tensor.matmul(
                            ps[:, :],
                            lhsT=wo_sb[:, p, d * 128:(d + 1) * 128],
                            rhs=zts[(p, j)][:, :],
                            start=(p == 0),
                            stop=(p == NPAIR - 1),
                        )
                    ot = opool.tile([128, 512], F32, name="ot")
                    nc.vector.tensor_copy(ot[:, :], ps[:, :])
                    nc.scalar.dma_start(
                        outT[d * 128:(d + 1) * 128, j * 512:(j + 1) * 512],
                        ot[:, :],
                    )

            # phase pipeline: pair-0 attention starts right after pair-0
            # projections + first V quarter; pair-1 projections and the
            # remaining V quarters fill PE while ACT runs pair-0 exps.
            qk_proj(0, [0, 1])
            qk_proj(0, [2, 3])
            v_proj([0, 1, 2, 3])
            v_proj([4, 5, 6, 7])
            attn(0, 0)
            attn(0, 1)
            v_proj([8, 9, 10, 11])
            v_proj([12, 13, 14, 15])
            qk_proj(1, [0, 1])
            qk_proj(1, [2, 3])
            attn(0, 2)
            attn(0, 3)
            attn(1, 0)
            attn(1, 1)
            emit_wo(0)
            attn(1, 2)
            emit_wo(1)
            attn(1, 3)
            emit_wo(2)
            emit_wo(3)

    nc.compile()
    return nc


_cache = {}


def _get(with_bias: bool):
    if with_bias not in _cache:
        _cache[with_bias] = _build(with_bias)
    return _cache[with_bias]


def kernel(x, W_Q, W_K, W_V, W_O, b_Q, b_K, b_V, b_O):
    global LAST_EXEC_NS
    x = np.asarray(x, dtype=np.float32)
    W_Q = np.asarray(W_Q, dtype=np.float32)
    W_K = np.asarray(W_K, dtype=np.float32)
    W_V = np.asarray(W_V, dtype=np.float32)
    W_O = np.asarray(W_O, dtype=np.float32)
    b_Q = np.asarray(b_Q, dtype=np.float32)
    b_K = np.asarray(b_K, dtype=np.float32)
    b_V = np.asarray(b_V, dtype=np.float32)
    b_O = np.asarray(b_O, dtype=np.float32)

    with_bias = bool(np.any(b_Q) or np.any(b_K) or np.any(b_V))
    nc = _get(with_bias)

    xT = np.ascontiguousarray(x.transpose(0, 2, 1))  # [B, DM, S]
    kp = np.arange(128)[:, None]
    qf = np.arange(128)[None, :]
    mask = np.where(qf >= kp, 1.0, 0.0).astype(ml_dtypes.bfloat16)

    in_maps = []
    for core in range(8):
        b, g = divmod(core, 4)
        hs = slice(NH * g, NH * g + NH)
        bf = ml_dtypes.bfloat16

        def packw(w):  # [DM, NH*DH] -> [128, DC*NH*DH] chunk-major
            return np.ascontiguousarray(
                w.reshape(DC, 128, NH * DH).transpose(1, 0, 2).reshape(128, DC * NH * DH)
            )

        m = {
            "xT": xT[b].astype(bf),
            "wq": packw((W_Q[hs] * 0.125).transpose(1, 0, 2).reshape(DM, NH * DH).astype(bf)),
            "wk": packw(W_K[hs].transpose(1, 0, 2).reshape(DM, NH * DH).astype(bf)),
            "wv": packw(W_V[hs].transpose(1, 0, 2).reshape(DM, NH * DH).astype(bf)),
            "wo": np.ascontiguousarray(
                W_O[hs].reshape(NH * DH, DM).astype(bf)
                .reshape(NPAIR, 128, DM).transpose(1, 0, 2).reshape(128, NPAIR * DM)
            ),
            "mask": mask,
        }
        if with_bias:
            m["bq"] = (b_Q[hs] * 0.125).reshape(1, NH * DH).astype(bf)
            m["bk"] = b_K[hs].reshape(1, NH * DH).astype(bf)
            m["bv"] = b_V[hs].reshape(1, NH * DH).astype(bf)
        in_maps.append(m)

    kwargs = {}
    if TRACE:
        kwargs = {"trace": True}
        if TRACE_DIR:
            kwargs["tmpdir"] = TRACE_DIR
    res = run_bass_kernel_spmd(nc, in_maps, list(range(8)), **kwargs)
    LAST_EXEC_NS = res.exec_time_ns

    out = np.empty((B, S, DM), dtype=np.float32)
    for b in range(B):
        acc = res.results[4 * b]["outT"].astype(np.float32)
        for g in range(1, 4):
            acc = acc + res.results[4 * b + g]["outT"]
        out[b] = acc.T + b_O[None, :]
    return out
